# revision 1
# baseline (speedup 1.0000x reference)
"""ColorAttention Trainium2 kernel.

Data-parallel over batch: core b handles batch element b.
Per core:
  - mask [256,256,313] f32 (82MB) is streamed from HBM and patch-reduced via
    block-diagonal ones matmuls on the PE (PSUM accumulation), giving
    m[s,c] = sum over 16x16 patch. Multiplicative attention mask
    is_one(m) = relu(1-(m-1)^2)  (exact for integer m: 1 iff m==1).
  - attention computed in transposed layout throughout:
      qkvT[f,n] = sum_e qkv_wT[e,f] * inputsT[e,n]
      scoresT[m,n] = sum_d kT[d,m] qT[d,n];  expT = exp(scoresT/tau) * mask
      outT_aug[d|1,n] = sum_m v_aug[m,d|1] expT[m,n]   (row 64 = denom)
      out[n,g] = (sum_{h,d} (outT_h/denom_h)[d,n] o_wT[h*64+d,g]) + o_b
  - all matmuls use float32r moving operands (1 cyc/col at N>=256, ~fp32 acc)
"""

import os
import numpy as np
import ml_dtypes

# tolerate environments without the optional NTFF profile hook module when
# BASS_TRACE is set externally
try:
    import antenv.axon_hooks  # noqa: F401
except Exception:
    import sys as _sys
    import types as _types
    _m = _types.ModuleType("antenv.axon_hooks")
    _m.set_axon_ntff_profile_hook = lambda h: None
    _m.get_axon_ntff_profile_hook = lambda: None
    try:
        import antenv
        antenv.axon_hooks = _m
        _sys.modules["antenv.axon_hooks"] = _m
    except Exception:
        pass

import concourse.bass as bass
import concourse.mybir as mybir
import concourse.tile as tile
from concourse import bacc
from concourse.bass_utils import run_bass_kernel_spmd

F32 = mybir.dt.float32
F32R = mybir.dt.float32r
BF16 = mybir.dt.bfloat16
AFT = mybir.ActivationFunctionType

B = 8
SEQ = 256
NCLS = 313
E = 512
NH = 8
HD = 64
N1 = SEQ + NCLS  # 569
P = 16
IMG = 256

# n/m chunking of the 569 token dim.
# fp32r matmuls require even free counts, so padded widths (..P) are used for
# fp32r operands/psum; real widths for bf16 ops and final stores.
N1P = 570
NCLSP = 314
CHUNKS = [(0, 128), (128, 128), (256, 128), (384, 128), (512, 57)]
CWP = [128, 128, 128, 128, 58]
SPANS = [(0, 512), (512, 58)]

LAST_RESULT = None
_CACHED = {}


def r32(ap):
    if ap.dtype == F32R:
        return ap
    return ap.bitcast(F32R)


def _build_program():
    nc = bacc.Bacc("TRN2", target_bir_lowering=False, debug=False, num_devices=B)

    # ---- DRAM I/O ----
    d_xT = nc.dram_tensor("xT", [E, N1], F32R, kind="ExternalInput").ap()
    d_mask = nc.dram_tensor("mask", [IMG, IMG * NCLS], F32, kind="ExternalInput").ap()
    d_qkvwT = nc.dram_tensor("qkv_wT", [E, 3 * E], F32R, kind="ExternalInput").ap()
    d_owT = nc.dram_tensor("o_wT", [E, E], F32R, kind="ExternalInput").ap()
    d_ob = nc.dram_tensor("o_b", [1, E], F32, kind="ExternalInput").ap()
    d_tau = nc.dram_tensor("tau", [1, 1], F32, kind="ExternalInput").ap()
    d_bd = nc.dram_tensor("bd", [16, 128, 128], BF16, kind="ExternalInput").ap()
    d_ident = nc.dram_tensor("ident", [128, 128], BF16, kind="ExternalInput").ap()
    d_unitv = nc.dram_tensor("unitv", [128, 512], F32R, kind="ExternalInput").ap()
    d_out = nc.dram_tensor("out", [N1, E], F32, kind="ExternalOutput").ap()

    with tile.TileContext(nc) as tc:
        _emit(nc, tc, d_xT, d_mask, d_qkvwT, d_owT, d_ob, d_tau, d_bd, d_ident, d_unitv, d_out)

    nc.compile()
    return nc


def _emit(nc, tc, d_xT, d_mask, d_qkvwT, d_owT, d_ob, d_tau, d_bd, d_ident, d_unitv, d_out):
    from contextlib import ExitStack

    ctx = ExitStack()
    singles = ctx.enter_context(tc.tile_pool(name="singles", bufs=1))
    expool = ctx.enter_context(tc.tile_pool(name="expT", bufs=40))
    opool = ctx.enter_context(tc.tile_pool(name="outTsb", bufs=8))
    spool = ctx.enter_context(tc.tile_pool(name="smalls", bufs=2))
    # scoped pools for the mask stream phase (closed before the tail phase
    # so their SBUF/PSUM space is reusable)
    ps_work = ctx.enter_context(tc.tile_pool(name="ps_work", bufs=4, space="PSUM"))
    mctx = ExitStack()
    mpool = mctx.enter_context(tc.tile_pool(name="mask_stream", bufs=3))
    ps_mask = mctx.enter_context(tc.tile_pool(name="ps_mask", bufs=2, space="PSUM"))

    # ---- persistent SBUF ----
    inputsT = [singles.tile([128, N1P], F32R, tag=f"inT{i}", name=f"inT{i}") for i in range(4)]
    qkvwT = [singles.tile([128, 3 * E], F32R, tag=f"qkvwT{i}", name=f"qkvwT{i}") for i in range(4)]
    owT = [singles.tile([64, E], F32R, tag=f"owT{i}", name=f"owT{i}") for i in range(8)]
    bd_sb = singles.tile([128, 16, 128], BF16, tag="bd", name="bd_sb")
    ident_sb = singles.tile([128, 128], BF16, tag="ident", name="ident_sb")
    ones_sb = singles.tile([128, 64], F32R, tag="ones", name="ones_sb")
    unitv_sb = singles.tile([128, 512], F32R, tag="unitv", name="unitv_sb")
    rtau = singles.tile([128, 1], F32, tag="rtau", name="rtau")
    ob_bc = singles.tile([128, E], F32, tag="ob", name="ob_bc")
    qkT = [singles.tile([128, N1P], F32R, tag=f"qkT{i}", name=f"qkT{i}") for i in range(8)]
    v_sb = [singles.tile([128, NH * (HD + 1)], BF16, tag=f"vsb{i}", name=f"v_sb{i}") for i in range(5)]
    isone = [singles.tile([128, NCLS], BF16, tag=f"iso{i}", name=f"isone{i}") for i in range(2)]
    isoT = [singles.tile([128, SEQ], BF16, tag=f"isoT{i}", name=f"isoT{i}") for i in range(3)]

    # ---- setup DMAs ----
    for i in range(4):
        nc.sync.dma_start(out=inputsT[i][:, :N1], in_=d_xT[i * 128:(i + 1) * 128, :])
        nc.vector.memset(inputsT[i][:, N1:N1P].bitcast(F32), 0.0)
        nc.sync.dma_start(out=qkvwT[i], in_=d_qkvwT[i * 128:(i + 1) * 128, :])
    for h in range(8):
        nc.sync.dma_start(out=owT[h], in_=d_owT[h * 64:(h + 1) * 64, :])
    for v in range(16):
        nc.sync.dma_start(out=bd_sb[:, v, :], in_=d_bd[v])
    nc.sync.dma_start(out=ident_sb, in_=d_ident)
    nc.sync.dma_start(out=unitv_sb, in_=d_unitv)
    nc.vector.memset(ones_sb[:].bitcast(F32), 1.0)
    # broadcast tau to all partitions (step-0 partition AP), then reciprocal
    tau_bc = bass.AP(tensor=d_tau.tensor, offset=d_tau.offset, ap=[[0, 128], [1, 1]])
    tau_sb = singles.tile([128, 1], F32, tag="tau", name="tau_sb")
    nc.gpsimd.dma_start(out=tau_sb, in_=tau_bc)
    nc.vector.reciprocal(out=rtau, in_=tau_sb)
    ob_src = bass.AP(tensor=d_ob.tensor, offset=d_ob.offset, ap=[[0, 128], [1, E]])
    nc.gpsimd.dma_start(out=ob_bc, in_=ob_src)
    neg1 = singles.tile([128, 1], F32, tag="neg1", name="neg1")
    nc.vector.memset(neg1, -1.0)

    # ---- HAM warmup: ~6us of dense dummy matmuls so the PE clock gate
    # opens (4/8 -> 8/8) before the mask stream starts; garbage in, garbage
    # to a scratch psum that is never read ----
    scr = singles.tile([128, 640], BF16, tag="scr", name="scr")
    nc.vector.memset(scr, 1.0)
    ps_warm = ps_work.tile([128, 512], F32, tag="pswork", name="ps_warm")
    for _ in range(20):
        nc.tensor.matmul(out=ps_warm, lhsT=scr[:, 0:128], rhs=scr[:, 128:640],
                         start=True, stop=True)

    # ---- attention work units (emitted interleaved with the mask stream) ----
    expT = {}

    def unit_qkvT(fc):
        def go():
            for sp, (s0, sw) in enumerate(SPANS):
                ps = ps_work.tile([128, sw], F32, tag="pswork", name="pswork")
                for ec in range(4):
                    nc.tensor.matmul(
                        out=ps,
                        lhsT=r32(qkvwT[ec][:, fc * 128:(fc + 1) * 128]),
                        rhs=r32(inputsT[ec][:, s0:s0 + sw]),
                        start=(ec == 0), stop=(ec == 3),
                    )
                nc.vector.tensor_copy(out=qkT[fc][:, s0:s0 + sw], in_=ps)
        return go

    def unit_v(mc):
        def go():
            c0, cw = CHUNKS[mc]
            cwp = CWP[mc]
            ps = ps_work.tile([128, E], F32, tag="pswork", name="pswork")
            for ec in range(4):
                nc.tensor.matmul(
                    out=ps[:cwp, :],
                    lhsT=r32(inputsT[ec][:, c0:c0 + cwp]),
                    rhs=r32(qkvwT[ec][:, 2 * E:3 * E]),
                    start=(ec == 0), stop=(ec == 3),
                )
            for h in range(NH):
                nc.vector.tensor_copy(
                    out=v_sb[mc][:cw, h * 65:h * 65 + 64],
                    in_=ps[:cw, h * 64:(h + 1) * 64],
                )
            nc.vector.memset(v_sb[mc][:cw, 64::65], 1.0)
        return go

    def unit_scores(h, mc):
        def go():
            c0, cw = CHUNKS[mc]
            cwp = CWP[mc]
            kt = qkT[4 + h // 2]
            qt = qkT[h // 2]
            hb = 64 * (h % 2)
            et = expool.tile([128, N1P], BF16, tag="expT", name="expT")
            expT[(h, mc)] = et
            for sp, (s0, sw) in enumerate(SPANS):
                ps = ps_work.tile([128, sw], F32, tag="pswork", name="pswork")
                nc.tensor.matmul(
                    out=ps[:cwp, :],
                    lhsT=r32(kt[hb:hb + 64, c0:c0 + cwp]),
                    rhs=r32(qt[hb:hb + 64, s0:s0 + sw]),
                    start=True, stop=True,
                )
                nc.scalar.activation(
                    out=et[:cwp, s0:s0 + sw], in_=ps[:cwp, :],
                    func=AFT.Exp, scale=rtau[:cwp],
                )
        return go

    def unit_mult(h, mc):
        def go():
            c0, cw = CHUNKS[mc]
            et = expT[(h, mc)]
            if mc < 2:
                nc.vector.tensor_mul(
                    out=et[:cw, SEQ:N1], in0=et[:cw, SEQ:N1], in1=isone[mc])
            else:
                nc.vector.tensor_mul(
                    out=et[:cw, 0:SEQ], in0=et[:cw, 0:SEQ], in1=isoT[mc - 2][:cw, :])
        return go

    units = [unit_qkvT(fc) for fc in range(8)]
    units += [unit_v(mc) for mc in range(5)]
    units += [unit_scores(h, mc) for h in range(NH) for mc in range(5)]

    # ---- is_one computation (psum -> multiplicative mask) ----
    ps_m = [None, None]

    def emit_isone(i):
        tmp = spool.tile([128, NCLS], F32, tag="isotmp", name="isotmp")
        nc.scalar.activation(out=tmp, in_=ps_m[i], func=AFT.Square, bias=neg1)
        nc.scalar.activation(out=isone[i], in_=tmp, func=AFT.Relu, scale=-1.0, bias=1.0)

    # ---- the mask stream: 32 tiles of [128 rows, 16q x 313c] (2.56MB) ----
    # 20KB contiguous per-partition source lines amortize DMA descriptor and
    # trigger overhead; each tile covers exactly one w, so its 16 sub-matmuls
    # share one stationary bd variant.
    ROWS_PER_TILE = 128
    QO = 16
    n_oct = IMG // QO  # 16
    ui = 0
    for rt in range(2):
        ps_m[rt] = ps_mask.tile([128, NCLS], F32, tag="psmask", name="psmask")
        for Q in range(n_oct):
            t = mpool.tile([128, QO * NCLS], BF16, tag="mstream", name="mstream")
            src = bass.AP(
                tensor=d_mask.tensor,
                offset=d_mask.offset + rt * ROWS_PER_TILE * IMG * NCLS + Q * QO * NCLS,
                ap=[[IMG * NCLS, 128], [1, QO * NCLS]],
            )
            nc.gpsimd.dma_start(out=t, in_=src)
            w = Q
            for j in range(QO):
                nc.tensor.matmul(
                    out=ps_m[rt],
                    lhsT=bd_sb[:, w, :],
                    rhs=t[:, j * NCLS:(j + 1) * NCLS],
                    start=(Q == 0 and j == 0),
                    stop=(Q == n_oct - 1 and j == QO - 1),
                )
            ti = rt * n_oct + Q
            if ti == 2:
                # dense PE burst to un-throttle the HAM early
                while ui < 4:
                    units[ui]()
                    ui += 1
            elif ti > 2 and ui < len(units):
                for _ in range(2):
                    if ui < len(units):
                        units[ui]()
                        ui += 1
        emit_isone(rt)
    while ui < len(units):
        units[ui]()
        ui += 1
    mctx.close()
    ps_out = ctx.enter_context(tc.tile_pool(name="ps_out", bufs=1, space="PSUM"))

    # ---- transpose is_one -> isoT (c on partitions) ----
    for i in range(2):
        for j in range(3):
            cw = 57 if j == 2 else 128
            pst = ps_work.tile([128, 128], BF16, tag="pswork", name="pswork_t")
            nc.tensor.transpose(out=pst[:cw, :], in_=isone[i][:, j * 128:j * 128 + cw],
                                identity=ident_sb)
            nc.vector.tensor_copy(out=isoT[j][:cw, i * 128:(i + 1) * 128], in_=pst[:cw, :])

    # ---- mask-mult + attn@v with gathered denominators ----
    # Per group of 4 heads: mask-mult expT, attn@v into psum (ones column of
    # v gives the softmax denominator in row 64), evacuate the unnormalized
    # outT to SBUF, and gather the 4 heads' denominator rows at partitions
    # {0,32,64,96} of a shared psum tile via K=1 unit-vector matmuls. Then a
    # single reciprocal per span serves the whole group; PE broadcasts each
    # head's reciprocal row and DVE normalizes outT in place.
    outT = [opool.tile([64, N1P], F32R, tag="outT", name="outT") for _ in range(NH)]
    for g in range(2):
        den_ps = {}
        for sp, (s0, sw) in enumerate(SPANS):
            den_ps[sp] = ps_out.tile([128, sw], F32, tag=f"denps{sp}", name="denps", bufs=1)
        for h4 in range(4):
            h = g * 4 + h4
            for mc in range(5):
                unit_mult(h, mc)()
            rec = spool.tile([65, N1P], F32R, tag="rec", name="rec")
            for sp, (s0, sw) in enumerate(SPANS):
                pso = ps_out.tile([65, sw], F32, tag="psout", name="psout", bufs=2)
                for mc in range(5):
                    c0, cw = CHUNKS[mc]
                    nc.tensor.matmul(
                        out=pso,
                        lhsT=v_sb[mc][:cw, h * 65:(h + 1) * 65],
                        rhs=expT[(h, mc)][:cw, s0:s0 + sw],
                        start=(mc == 0), stop=(mc == 4),
                    )
                with nc.allow_low_precision(reason="f32r copies"):
                    nc.scalar.activation(out=rec[64:65, s0:s0 + sw], in_=pso[64:65, :],
                                         func=AFT.Copy)
                    nc.vector.tensor_copy(out=outT[h][:, s0:s0 + sw], in_=pso[0:64, :])
                nc.tensor.matmul(
                    out=den_ps[sp],
                    lhsT=r32(unitv_sb[64:65, h4 * 128:(h4 + 1) * 128]),
                    rhs=r32(rec[64:65, s0:s0 + sw]),
                    start=(h4 == 0), stop=(h4 == 3),
                )
        drec = {}
        for sp, (s0, sw) in enumerate(SPANS):
            dr = spool.tile([128, sw], F32R, tag=f"drec{sp}", name=f"drec{sp}")
            with nc.allow_low_precision(reason="f32r reciprocal"):
                nc.vector.reciprocal(out=dr, in_=den_ps[sp])
            drec[sp] = dr
        for h4 in range(4):
            h = g * 4 + h4
            bc_sb = spool.tile([64, N1P], F32, tag="bcsb", name="bcsb")
            for sp, (s0, sw) in enumerate(SPANS):
                psb = ps_work.tile([64, sw], F32, tag="pswork", name="psb")
                nc.tensor.matmul(
                    out=psb,
                    lhsT=r32(ones_sb[32 * h4:32 * h4 + 1, :]),
                    rhs=drec[sp][32 * h4:32 * h4 + 1, :],
                    start=True, stop=True,
                    tile_position=(32 * h4, 0),
                )
                nc.scalar.activation(out=bc_sb[:, s0:s0 + sw], in_=psb, func=AFT.Copy)
                with nc.allow_low_precision(reason="in-place normalize"):
                    nc.vector.tensor_mul(
                        out=outT[h][:, s0:s0 + sw], in0=outT[h][:, s0:s0 + sw],
                        in1=bc_sb[:, s0:s0 + sw])

    # ---- o_proj + bias + store ----
    for mc in range(5):
        c0, cw = CHUNKS[mc]
        cwp = CWP[mc]
        psf = ps_work.tile([128, E], F32, tag="pswork", name="psf")
        for h in range(NH):
            nc.tensor.matmul(
                out=psf[:cwp, :],
                lhsT=r32(outT[h][:, c0:c0 + cwp]),
                rhs=r32(owT[h]),
                start=(h == 0), stop=(h == NH - 1),
            )
        fin = spool.tile([128, E], F32, tag="fin", name="fin")
        nc.vector.tensor_add(out=fin[:cw, :], in0=psf[:cw, :], in1=ob_bc[:cw, :])
        nc.sync.dma_start(out=d_out[c0:c0 + cw, :], in_=fin[:cw, :])

    ctx.close()


def _constants():
    # block-diag: bd[w][r, s'] = 1 iff s' == (r//16)*16 + w
    bd = np.zeros((16, 128, 128), dtype=np.float32)
    r = np.arange(128)
    for w in range(16):
        bd[w, r, (r // 16) * 16 + w] = 1.0
    ident = np.eye(128, dtype=ml_dtypes.bfloat16)
    unitv = np.zeros((128, 512), dtype=np.float32)
    for h4 in range(4):
        unitv[:, h4 * 128 + 32 * h4] = 1.0
    return bd.astype(ml_dtypes.bfloat16), ident, unitv


def kernel(x, colors, mask, qkv_w, o_w, o_b, tau):
    global LAST_RESULT
    if "nc" not in _CACHED:
        _CACHED["nc"] = _build_program()
    nc = _CACHED["nc"]

    bd, ident, unitv = _constants()
    qkv_wT = np.ascontiguousarray(np.asarray(qkv_w, dtype=np.float32).T)
    o_wT = np.ascontiguousarray(np.asarray(o_w, dtype=np.float32).T)
    o_b2 = np.asarray(o_b, dtype=np.float32).reshape(1, E)
    tau2 = np.asarray(tau, dtype=np.float32).reshape(1, 1)

    in_maps = []
    for b in range(B):
        xT = np.ascontiguousarray(
            np.concatenate([np.asarray(x[b]), np.asarray(colors[b])], axis=0).T
        ).astype(np.float32)
        mb = np.ascontiguousarray(np.asarray(mask[b], dtype=np.float32)).reshape(IMG, IMG * NCLS)
        in_maps.append({
            "xT": xT, "mask": mb, "qkv_wT": qkv_wT, "o_wT": o_wT,
            "o_b": o_b2, "tau": tau2, "bd": bd, "ident": ident, "unitv": unitv,
        })

    res = run_bass_kernel_spmd(nc, in_maps, list(range(B)))
    LAST_RESULT = res
    out = np.stack([res.results[i]["out"] for i in range(B)]).astype(np.float32)
    return out



# revision 7
# speedup vs baseline: 1.7190x; 1.7190x over previous
"""ColorAttention Trainium2 kernel.

Data-parallel over batch: core b handles batch element b.
Per core:
  - mask [256,256,313] f32 (82MB) is streamed from HBM and patch-reduced via
    block-diagonal ones matmuls on the PE (PSUM accumulation), giving
    m[s,c] = sum over 16x16 patch. Multiplicative attention mask
    is_one(m) = relu(1-(m-1)^2)  (exact for integer m: 1 iff m==1).
  - attention computed in transposed layout throughout:
      qkvT[f,n] = sum_e qkv_wT[e,f] * inputsT[e,n]
      scoresT[m,n] = sum_d kT[d,m] qT[d,n];  expT = exp(scoresT/tau) * mask
      outT_aug[d|1,n] = sum_m v_aug[m,d|1] expT[m,n]   (row 64 = denom)
      out[n,g] = (sum_{h,d} (outT_h/denom_h)[d,n] o_wT[h*64+d,g]) + o_b
  - all matmuls use float32r moving operands (1 cyc/col at N>=256, ~fp32 acc)
"""

import os
import numpy as np
import ml_dtypes

# tolerate environments without the optional NTFF profile hook module when
# BASS_TRACE is set externally
try:
    import antenv.axon_hooks  # noqa: F401
except Exception:
    import sys as _sys
    import types as _types
    _m = _types.ModuleType("antenv.axon_hooks")
    _m.set_axon_ntff_profile_hook = lambda h: None
    _m.get_axon_ntff_profile_hook = lambda: None
    try:
        import antenv
        antenv.axon_hooks = _m
        _sys.modules["antenv.axon_hooks"] = _m
    except Exception:
        pass

import concourse.bass as bass
import concourse.mybir as mybir
import concourse.tile as tile
from concourse import bacc
from concourse.bass_utils import run_bass_kernel_spmd

F32 = mybir.dt.float32
F32R = mybir.dt.float32r
BF16 = mybir.dt.bfloat16
FP8 = mybir.dt.float8e4
AFT = mybir.ActivationFunctionType

B = 8
SEQ = 256
NCLS = 313
E = 512
NH = 8
HD = 64
N1 = SEQ + NCLS  # 569
P = 16
IMG = 256

# n/m chunking of the 569 token dim.
# fp32r matmuls require even free counts, so padded widths (..P) are used for
# fp32r operands/psum; real widths for bf16 ops and final stores.
N1P = 570
NCLSP = 314
CHUNKS = [(0, 128), (128, 128), (256, 128), (384, 128), (512, 57)]
CWP = [128, 128, 128, 128, 58]
SPANS = [(0, 512), (512, 58)]

LAST_RESULT = None
_CACHED = {}


def r32(ap):
    if ap.dtype == F32R:
        return ap
    return ap.bitcast(F32R)


def _build_program():
    nc = bacc.Bacc("TRN2", target_bir_lowering=False, debug=False, num_devices=B)

    # ---- DRAM I/O ----
    d_xT = nc.dram_tensor("xT", [E, N1], F32R, kind="ExternalInput").ap()
    d_mask = nc.dram_tensor("mask", [IMG, IMG * NCLS], FP8, kind="ExternalInput").ap()
    d_qkvwT = nc.dram_tensor("qkv_wT", [E, 3 * E], F32R, kind="ExternalInput").ap()
    d_owT = nc.dram_tensor("o_wT", [E, E], F32R, kind="ExternalInput").ap()
    d_ob = nc.dram_tensor("o_b", [1, E], F32, kind="ExternalInput").ap()
    d_tau = nc.dram_tensor("tau", [1, 1], F32, kind="ExternalInput").ap()
    d_bd = nc.dram_tensor("bd", [16, 128, 128], FP8, kind="ExternalInput").ap()
    d_ident = nc.dram_tensor("ident", [128, 128], BF16, kind="ExternalInput").ap()
    d_unitv = nc.dram_tensor("unitv", [128, 512], F32R, kind="ExternalInput").ap()
    d_out = nc.dram_tensor("out", [N1, E], F32, kind="ExternalOutput").ap()

    with tile.TileContext(nc) as tc:
        _emit(nc, tc, d_xT, d_mask, d_qkvwT, d_owT, d_ob, d_tau, d_bd, d_ident, d_unitv, d_out)

    nc.compile()
    return nc


def _emit(nc, tc, d_xT, d_mask, d_qkvwT, d_owT, d_ob, d_tau, d_bd, d_ident, d_unitv, d_out):
    from contextlib import ExitStack

    ctx = ExitStack()
    singles = ctx.enter_context(tc.tile_pool(name="singles", bufs=1))
    expool = ctx.enter_context(tc.tile_pool(name="expT", bufs=40))
    opool = ctx.enter_context(tc.tile_pool(name="outTsb", bufs=8))
    spool = ctx.enter_context(tc.tile_pool(name="smalls", bufs=2))
    # scoped pools for the mask stream phase (closed before the tail phase
    # so their SBUF/PSUM space is reusable)
    ps_work = ctx.enter_context(tc.tile_pool(name="ps_work", bufs=4, space="PSUM"))
    mctx = ExitStack()
    mpool = mctx.enter_context(tc.tile_pool(name="mask_stream", bufs=3))
    ps_mask = mctx.enter_context(tc.tile_pool(name="ps_mask", bufs=2, space="PSUM"))

    # ---- persistent SBUF ----
    inputsT = [singles.tile([128, N1P], F32R, tag=f"inT{i}", name=f"inT{i}") for i in range(4)]
    qkvwT = [singles.tile([128, 3 * E], F32R, tag=f"qkvwT{i}", name=f"qkvwT{i}") for i in range(4)]
    owT = [singles.tile([64, E], F32R, tag=f"owT{i}", name=f"owT{i}") for i in range(8)]
    bd_sb = singles.tile([128, 16, 128], FP8, tag="bd", name="bd_sb")
    ident_sb = singles.tile([128, 128], BF16, tag="ident", name="ident_sb")
    ones_sb = singles.tile([128, 64], F32R, tag="ones", name="ones_sb")
    unitv_sb = singles.tile([128, 512], F32R, tag="unitv", name="unitv_sb")
    rtau = singles.tile([128, 1], F32, tag="rtau", name="rtau")
    ob_bc = singles.tile([128, E], F32, tag="ob", name="ob_bc")
    qkT = [singles.tile([128, N1P], F32R, tag=f"qkT{i}", name=f"qkT{i}") for i in range(8)]
    v_sb = [singles.tile([128, NH * (HD + 1)], BF16, tag=f"vsb{i}", name=f"v_sb{i}") for i in range(5)]
    isone = [singles.tile([128, NCLS], BF16, tag=f"iso{i}", name=f"isone{i}") for i in range(2)]
    isoT = [singles.tile([128, SEQ], BF16, tag=f"isoT{i}", name=f"isoT{i}") for i in range(3)]

    # ---- setup DMAs ----
    for i in range(4):
        nc.sync.dma_start(out=inputsT[i][:, :N1], in_=d_xT[i * 128:(i + 1) * 128, :])
        nc.vector.memset(inputsT[i][:, N1:N1P].bitcast(F32), 0.0)
        nc.sync.dma_start(out=qkvwT[i], in_=d_qkvwT[i * 128:(i + 1) * 128, :])
    for h in range(8):
        nc.sync.dma_start(out=owT[h], in_=d_owT[h * 64:(h + 1) * 64, :])
    for v in range(16):
        nc.sync.dma_start(out=bd_sb[:, v, :], in_=d_bd[v])
    nc.sync.dma_start(out=ident_sb, in_=d_ident)
    nc.sync.dma_start(out=unitv_sb, in_=d_unitv)
    nc.vector.memset(ones_sb[:].bitcast(F32), 1.0)
    # broadcast tau to all partitions (step-0 partition AP), then reciprocal
    tau_bc = bass.AP(tensor=d_tau.tensor, offset=d_tau.offset, ap=[[0, 128], [1, 1]])
    tau_sb = singles.tile([128, 1], F32, tag="tau", name="tau_sb")
    nc.gpsimd.dma_start(out=tau_sb, in_=tau_bc)
    nc.vector.reciprocal(out=rtau, in_=tau_sb)
    ob_src = bass.AP(tensor=d_ob.tensor, offset=d_ob.offset, ap=[[0, 128], [1, E]])
    nc.gpsimd.dma_start(out=ob_bc, in_=ob_src)
    neg1 = singles.tile([128, 1], F32, tag="neg1", name="neg1")
    nc.vector.memset(neg1, -1.0)

    # ---- HAM warmup: ~6us of dense dummy matmuls so the PE clock gate
    # opens (4/8 -> 8/8) before the mask stream starts; garbage in, garbage
    # to a scratch psum that is never read ----
    scr = singles.tile([128, 640], BF16, tag="scr", name="scr")
    nc.vector.memset(scr, 1.0)
    ps_warm = ps_work.tile([128, 512], F32, tag="pswork", name="ps_warm")
    for _ in range(20):
        nc.tensor.matmul(out=ps_warm, lhsT=scr[:, 0:128], rhs=scr[:, 128:640],
                         start=True, stop=True)

    # ---- attention work units (emitted interleaved with the mask stream) ----
    expT = {}

    def unit_qkvT(fc):
        def go():
            for sp, (s0, sw) in enumerate(SPANS):
                ps = ps_work.tile([128, sw], F32, tag="pswork", name="pswork")
                for ec in range(4):
                    nc.tensor.matmul(
                        out=ps,
                        lhsT=r32(qkvwT[ec][:, fc * 128:(fc + 1) * 128]),
                        rhs=r32(inputsT[ec][:, s0:s0 + sw]),
                        start=(ec == 0), stop=(ec == 3),
                    )
                nc.vector.tensor_copy(out=qkT[fc][:, s0:s0 + sw], in_=ps)
        return go

    def unit_v(mc):
        def go():
            c0, cw = CHUNKS[mc]
            cwp = CWP[mc]
            ps = ps_work.tile([128, E], F32, tag="pswork", name="pswork")
            for ec in range(4):
                nc.tensor.matmul(
                    out=ps[:cwp, :],
                    lhsT=r32(inputsT[ec][:, c0:c0 + cwp]),
                    rhs=r32(qkvwT[ec][:, 2 * E:3 * E]),
                    start=(ec == 0), stop=(ec == 3),
                )
            for h in range(NH):
                nc.vector.tensor_copy(
                    out=v_sb[mc][:cw, h * 65:h * 65 + 64],
                    in_=ps[:cw, h * 64:(h + 1) * 64],
                )
            nc.vector.memset(v_sb[mc][:cw, 64::65], 1.0)
        return go

    def unit_scores(h, mc):
        def go():
            c0, cw = CHUNKS[mc]
            cwp = CWP[mc]
            kt = qkT[4 + h // 2]
            qt = qkT[h // 2]
            hb = 64 * (h % 2)
            et = expool.tile([128, N1P], BF16, tag="expT", name="expT")
            expT[(h, mc)] = et
            for sp, (s0, sw) in enumerate(SPANS):
                ps = ps_work.tile([128, sw], F32, tag="pswork", name="pswork")
                nc.tensor.matmul(
                    out=ps[:cwp, :],
                    lhsT=r32(kt[hb:hb + 64, c0:c0 + cwp]),
                    rhs=r32(qt[hb:hb + 64, s0:s0 + sw]),
                    start=True, stop=True,
                )
                nc.scalar.activation(
                    out=et[:cwp, s0:s0 + sw], in_=ps[:cwp, :],
                    func=AFT.Exp, scale=rtau[:cwp],
                )
        return go

    def unit_mult(h, mc):
        def go():
            c0, cw = CHUNKS[mc]
            et = expT[(h, mc)]
            if mc < 2:
                nc.vector.tensor_mul(
                    out=et[:cw, SEQ:N1], in0=et[:cw, SEQ:N1], in1=isone[mc])
            else:
                nc.vector.tensor_mul(
                    out=et[:cw, 0:SEQ], in0=et[:cw, 0:SEQ], in1=isoT[mc - 2][:cw, :])
        return go

    units = [unit_qkvT(fc) for fc in range(8)]
    units += [unit_v(mc) for mc in range(5)]
    units += [unit_scores(h, mc) for h in range(NH) for mc in range(5)]

    # ---- is_one computation (psum -> multiplicative mask) ----
    ps_m = [None, None]

    def emit_isone(i):
        tmp = spool.tile([128, NCLS], F32, tag="isotmp", name="isotmp")
        nc.scalar.activation(out=tmp, in_=ps_m[i], func=AFT.Square, bias=neg1)
        nc.scalar.activation(out=isone[i], in_=tmp, func=AFT.Relu, scale=-1.0, bias=1.0)

    # ---- the mask stream: 16 fp8 tiles of [128 rows, 2w x 16q x 313c]
    # (1.25MB) ----
    # fp8 on the host side makes the DMA a pure HWDGE copy at 1B/elem;
    # each tile covers two w groups, each w's 16 sub-matmuls share one
    # stationary bd variant.
    ROWS_PER_TILE = 128
    QO = 16
    n_tiles = 8  # per row-block; each tile spans 2 w groups
    ui = 0
    for rt in range(2):
        ps_m[rt] = ps_mask.tile([128, NCLS], F32, tag="psmask", name="psmask")
        for T in range(n_tiles):
            t = mpool.tile([128, 2, QO * NCLS], FP8, tag="mstream", name="mstream")
            src = bass.AP(
                tensor=d_mask.tensor,
                offset=d_mask.offset + rt * ROWS_PER_TILE * IMG * NCLS
                + T * 2 * QO * NCLS,
                ap=[[IMG * NCLS, 128], [1, 2 * QO * NCLS]],
            )
            nc.sync.dma_start(out=t, in_=src)
            for wi in range(2):
                w = 2 * T + wi
                for j in range(QO):
                    nc.tensor.matmul(
                        out=ps_m[rt],
                        lhsT=bd_sb[:, w, :],
                        rhs=t[:, wi, j * NCLS:(j + 1) * NCLS],
                        start=(T == 0 and wi == 0 and j == 0),
                        stop=(T == n_tiles - 1 and wi == 1 and j == QO - 1),
                    )
            ti = rt * n_tiles + T
            if ti == 1:
                # dense PE burst to un-throttle the HAM early
                while ui < 4:
                    units[ui]()
                    ui += 1
            elif ti > 1 and ui < len(units):
                for _ in range(3):
                    if ui < len(units):
                        units[ui]()
                        ui += 1
        emit_isone(rt)
    while ui < len(units):
        units[ui]()
        ui += 1
    mctx.close()
    ps_out = ctx.enter_context(tc.tile_pool(name="ps_out", bufs=1, space="PSUM"))

    # ---- transpose is_one -> isoT (c on partitions) ----
    for i in range(2):
        for j in range(3):
            cw = 57 if j == 2 else 128
            pst = ps_work.tile([128, 128], BF16, tag="pswork", name="pswork_t")
            nc.tensor.transpose(out=pst[:cw, :], in_=isone[i][:, j * 128:j * 128 + cw],
                                identity=ident_sb)
            nc.vector.tensor_copy(out=isoT[j][:cw, i * 128:(i + 1) * 128], in_=pst[:cw, :])

    # ---- mask-mult + attn@v with gathered denominators ----
    # Per group of 4 heads: mask-mult expT, attn@v into psum (ones column of
    # v gives the softmax denominator in row 64), evacuate the unnormalized
    # outT to SBUF, and gather the 4 heads' denominator rows at partitions
    # {0,32,64,96} of a shared psum tile via K=1 unit-vector matmuls. Then a
    # single reciprocal per span serves the whole group; PE broadcasts each
    # head's reciprocal row and DVE normalizes outT in place.
    outT = [opool.tile([64, N1P], F32R, tag="outT", name="outT") for _ in range(NH)]
    for g in range(2):
        den_ps = {}
        for sp, (s0, sw) in enumerate(SPANS):
            den_ps[sp] = ps_out.tile([128, sw], F32, tag=f"denps{sp}", name="denps", bufs=1)
        for h4 in range(4):
            h = g * 4 + h4
            for mc in range(5):
                unit_mult(h, mc)()
            rec = spool.tile([65, N1P], F32R, tag="rec", name="rec")
            for sp, (s0, sw) in enumerate(SPANS):
                pso = ps_out.tile([65, sw], F32, tag="psout", name="psout", bufs=2)
                for mc in range(5):
                    c0, cw = CHUNKS[mc]
                    nc.tensor.matmul(
                        out=pso,
                        lhsT=v_sb[mc][:cw, h * 65:(h + 1) * 65],
                        rhs=expT[(h, mc)][:cw, s0:s0 + sw],
                        start=(mc == 0), stop=(mc == 4),
                    )
                with nc.allow_low_precision(reason="f32r copies"):
                    nc.scalar.activation(out=rec[64:65, s0:s0 + sw], in_=pso[64:65, :],
                                         func=AFT.Copy)
                    nc.vector.tensor_copy(out=outT[h][:, s0:s0 + sw], in_=pso[0:64, :])
                nc.tensor.matmul(
                    out=den_ps[sp],
                    lhsT=r32(unitv_sb[64:65, h4 * 128:(h4 + 1) * 128]),
                    rhs=r32(rec[64:65, s0:s0 + sw]),
                    start=(h4 == 0), stop=(h4 == 3),
                )
        drec = {}
        for sp, (s0, sw) in enumerate(SPANS):
            dr = spool.tile([128, sw], F32R, tag=f"drec{sp}", name=f"drec{sp}")
            with nc.allow_low_precision(reason="f32r reciprocal"):
                nc.vector.reciprocal(out=dr, in_=den_ps[sp])
            drec[sp] = dr
        for h4 in range(4):
            h = g * 4 + h4
            bc_sb = spool.tile([64, N1P], F32, tag="bcsb", name="bcsb")
            for sp, (s0, sw) in enumerate(SPANS):
                psb = ps_work.tile([64, sw], F32, tag="pswork", name="psb")
                nc.tensor.matmul(
                    out=psb,
                    lhsT=r32(ones_sb[32 * h4:32 * h4 + 1, :]),
                    rhs=drec[sp][32 * h4:32 * h4 + 1, :],
                    start=True, stop=True,
                    tile_position=(32 * h4, 0),
                )
                nc.scalar.activation(out=bc_sb[:, s0:s0 + sw], in_=psb, func=AFT.Copy)
                with nc.allow_low_precision(reason="in-place normalize"):
                    nc.vector.tensor_mul(
                        out=outT[h][:, s0:s0 + sw], in0=outT[h][:, s0:s0 + sw],
                        in1=bc_sb[:, s0:s0 + sw])

    # ---- o_proj + bias + store ----
    for mc in range(5):
        c0, cw = CHUNKS[mc]
        cwp = CWP[mc]
        psf = ps_work.tile([128, E], F32, tag="pswork", name="psf")
        for h in range(NH):
            nc.tensor.matmul(
                out=psf[:cwp, :],
                lhsT=r32(outT[h][:, c0:c0 + cwp]),
                rhs=r32(owT[h]),
                start=(h == 0), stop=(h == NH - 1),
            )
        fin = spool.tile([128, E], F32, tag="fin", name="fin")
        nc.vector.tensor_add(out=fin[:cw, :], in0=psf[:cw, :], in1=ob_bc[:cw, :])
        nc.sync.dma_start(out=d_out[c0:c0 + cw, :], in_=fin[:cw, :])

    ctx.close()


def _constants():
    # block-diag: bd[w][r, s'] = 1 iff s' == (r//16)*16 + w
    bd = np.zeros((16, 128, 128), dtype=np.float32)
    r = np.arange(128)
    for w in range(16):
        bd[w, r, (r // 16) * 16 + w] = 1.0
    ident = np.eye(128, dtype=ml_dtypes.bfloat16)
    unitv = np.zeros((128, 512), dtype=np.float32)
    for h4 in range(4):
        unitv[:, h4 * 128 + 32 * h4] = 1.0
    return bd.astype(ml_dtypes.float8_e4m3), ident, unitv


def kernel(x, colors, mask, qkv_w, o_w, o_b, tau):
    global LAST_RESULT
    if "nc" not in _CACHED:
        _CACHED["nc"] = _build_program()
    nc = _CACHED["nc"]

    bd, ident, unitv = _constants()
    qkv_wT = np.ascontiguousarray(np.asarray(qkv_w, dtype=np.float32).T)
    o_wT = np.ascontiguousarray(np.asarray(o_w, dtype=np.float32).T)
    o_b2 = np.asarray(o_b, dtype=np.float32).reshape(1, E)
    tau2 = np.asarray(tau, dtype=np.float32).reshape(1, 1)

    in_maps = []
    for b in range(B):
        xT = np.ascontiguousarray(
            np.concatenate([np.asarray(x[b]), np.asarray(colors[b])], axis=0).T
        ).astype(np.float32)
        # mask values are exactly 0.0/1.0 -> cast to fp8 is lossless and
        # quarters the HBM stream
        mb = np.asarray(mask[b], dtype=np.float32).astype(
            ml_dtypes.float8_e4m3).reshape(IMG, IMG * NCLS)
        in_maps.append({
            "xT": xT, "mask": mb, "qkv_wT": qkv_wT, "o_wT": o_wT,
            "o_b": o_b2, "tau": tau2, "bd": bd, "ident": ident, "unitv": unitv,
        })

    res = run_bass_kernel_spmd(nc, in_maps, list(range(B)))
    LAST_RESULT = res
    out = np.stack([res.results[i]["out"] for i in range(B)]).astype(np.float32)
    return out

